# revision 2
# baseline (speedup 1.0000x reference)
"""Trainium2 Bass kernel for nn_Autoencoder__gen204 (8-core data parallel), v2.

Same math as v1 (quantum circuit collapsed to phi @ W81 on host), restructured
DMA plan to approach the per-core HBM roofline (~26MB traffic / ~358 GB/s):

- x is cast to bf16 ON HOST and repacked chunk-major-contiguous, so the
  read stream is 8.4MB instead of 16.8 (f32) and each half-chunk piece is
  one fully contiguous ~1MB DMA (8KB per partition row). Measured: this
  DMA shape sustains ~415 GB/s per core.
- x rides the sync HWDGE queue: the sync engine has no compute duties,
  so its ~0.7us-per-trigger dma_start instructions never block compute
  (a scalar-queue x stream deadlocks evictions behind the triggers; a
  gpsimd x stream delays the out-DMAs).
- all matmuls are bf16 (f32r measured ~1.7x slower per MM despite the
  cost model: fp32_mode=HIGH, 628-795ns vs 379ns at equal clock).
- small weights ride the scalar queue at program start (7 quick
  triggers, drained before the first eviction needs the scalar engine).
- all output DMAs go on the gpsimd (SWDGE) queue so writes never queue
  behind reads on the HWDGE ring; writes start as soon as dec(0) runs.
"""

import ml_dtypes
import numpy as np

import concourse.bass as bass
import concourse.mybir as mybir
import concourse.tile as tile
from concourse import bacc
from concourse.bass_utils import run_bass_kernel_spmd

# ----- problem constants (hardcoded per contract) -----
B, D, H1, H2, L = 16384, 2048, 128, 64, 32
NQ, NL = 4, 3
NCORES = 8
BL = B // NCORES  # 2048 batch per core
P = 128
KD = D // P  # 16 k-chunks for the D contraction
NCH = 4  # batch chunks per core
CW = BL // NCH  # 512 cols per chunk
F32 = mybir.dt.float32
F32R = mybir.dt.float32r
BF16 = mybir.dt.bfloat16
F16 = mybir.dt.float16

# =====================================================================
# Host-side quantum-circuit collapse: qw -> W81 (81, 4)
# =====================================================================

_I2 = np.eye(2, dtype=np.complex128)
_SY = np.array([[0, -1j], [1j, 0]], dtype=np.complex128)
_SZ = np.array([[1, 0], [0, -1]], dtype=np.complex128)
_CNOT4 = np.array(
    [[1, 0, 0, 0], [0, 1, 0, 0], [0, 0, 0, 1], [0, 0, 1, 0]], dtype=np.complex128
).reshape(2, 2, 2, 2)
_bits = (np.arange(2**NQ)[:, None] >> np.arange(NQ - 1, -1, -1)) & 1
_Z_SIGNS = (1 - 2 * _bits).astype(np.float64)  # (16, 4)


def _rot_mat(phi, theta, omega):
    ez = np.exp(-0.5j * phi)
    rz1 = np.array([[ez, 0], [0, np.conj(ez)]], dtype=np.complex128)
    c, s = np.cos(theta / 2), np.sin(theta / 2)
    ry = np.array([[c, -s], [s, c]], dtype=np.complex128)
    eo = np.exp(-0.5j * omega)
    rz2 = np.array([[eo, 0], [0, np.conj(eo)]], dtype=np.complex128)
    return rz2 @ ry @ rz1


def _apply1(state, U, wire):
    state = np.tensordot(U, state, axes=[[1], [wire]])
    return np.moveaxis(state, 0, wire)


def _apply_cnot(state, c, t):
    state = np.tensordot(_CNOT4, state, axes=[[2, 3], [c, t]])
    return np.moveaxis(state, [0, 1], [c, t])


def _w81_from_qw(qw):
    qw = np.asarray(qw, dtype=np.float64)
    V = np.eye(16, dtype=np.complex128).reshape(2, 2, 2, 2, 16)
    for layer in range(NL):
        for q in range(NQ):
            V = _apply1(V, _rot_mat(*qw[layer, q]), q)
        for q in range(NQ - 1):
            V = _apply_cnot(V, q, q + 1)
    V = V.reshape(16, 16)
    paulis = [_I2, _SY, _SZ]  # digit 0 -> I(1), 1 -> Y(sin), 2 -> Z(cos)
    W = np.zeros((81, NQ), dtype=np.float64)
    for q in range(NQ):
        O = V.conj().T @ (_Z_SIGNS[:, q][:, None] * V)
        for k in range(81):
            d = [(k // 27) % 3, (k // 9) % 3, (k // 3) % 3, k % 3]
            Pm = np.kron(
                np.kron(paulis[d[0]], paulis[d[1]]),
                np.kron(paulis[d[2]], paulis[d[3]]),
            )
            alpha = np.trace(O @ Pm) / 16.0
            n_y = sum(1 for x in d if x == 1)
            W[k, q] = alpha.real * ((-1) ** n_y)  # t(Y) = -sin; feature is +sin
    return W


def _sign_classes():
    """(40, 4) int: e in {-1,0,1}^4 \\ {0} mod global sign (first nonzero +1)."""
    E = []
    for a in (-1, 0, 1):
        for b in (-1, 0, 1):
            for c in (-1, 0, 1):
                for d in (-1, 0, 1):
                    e = (a, b, c, d)
                    if e == (0, 0, 0, 0):
                        continue
                    if next(x for x in e if x != 0) < 0:
                        continue
                    E.append(e)
    return np.array(E, dtype=np.float64)


def _fourier_mid(qw, enc_w2, enc_b2, dec_w0):
    """Rewrite refined = W81^T phi(t) in the Fourier basis
    G(t) = [1, cos(e.t), sin(e.t)]_e so the device computes the 81 features
    with ONE matmul (args) + ONE Sin activation instead of selector matmuls
    and elementwise products.  Returns (w2g [64,81], biasg [81], wfg [81,64]).
    """
    w81 = _w81_from_qw(np.asarray(qw, np.float64))  # (81, NQ)
    E = _sign_classes()  # (40, 4)

    def phi_row(t):
        fs = np.stack([np.ones(NQ), np.sin(t), np.cos(t)])
        out = np.empty(81)
        for k in range(81):
            d = [(k // 27) % 3, (k // 9) % 3, (k // 3) % 3, k % 3]
            out[k] = np.prod([fs[d[q], q] for q in range(NQ)])
        return out

    def g_row(t):
        u = E @ t
        g = np.empty(81)
        g[0] = 1.0
        g[1::2] = np.cos(u)
        g[2::2] = np.sin(u)
        return g

    rng = np.random.default_rng(12345)
    T = rng.normal(size=(400, NQ)) * 2.0
    GT = np.stack([g_row(t) for t in T])
    PT = np.stack([phi_row(t) for t in T])
    Wnew, *_ = np.linalg.lstsq(GT, PT @ w81, rcond=None)  # (81, NQ)

    w2q = np.asarray(enc_w2, np.float64)[:, :NQ]  # (64, 4)
    b2q = np.asarray(enc_b2, np.float64)[:NQ]
    cols = w2q @ E.T  # (64, 40)
    w2g = np.zeros((H2, 81))
    w2g[:, 1::2] = cols
    w2g[:, 2::2] = cols
    ub = E @ b2q  # (40,)
    biasg = np.zeros(81)
    biasg[0] = np.pi / 2
    biasg[1::2] = ub + np.pi / 2  # sin(u + pi/2) = cos(u)
    biasg[2::2] = ub
    wfg = Wnew @ np.asarray(dec_w0, np.float64)  # (81, 64)
    return (
        w2g.astype(np.float32),
        biasg.astype(np.float32),
        wfg.astype(np.float32),
    )


# =====================================================================
# Bass program (one core; identical across the 8 cores)
# =====================================================================

_PROGRAM_CACHE = {}


def _build_program(debug=False, dec2_bias_zero=False, warmup=12, warmup2=6):
    key = (debug, dec2_bias_zero, warmup, warmup2)
    if key in _PROGRAM_CACHE:
        return _PROGRAM_CACHE[key]

    nc = bacc.Bacc("TRN2", target_bir_lowering=False, debug=debug)

    # xin row n*128+p, col k*512+c  =  x^T[k*128+p, n*512+c]: host-repacked
    # bf16 so each half-chunk piece [128, 4096] is one fully contiguous
    # ~1MB DMA (8KB per partition row) -> few triggers, max HBM efficiency.
    xin = nc.dram_tensor("xin", [NCH * P, KD * CW], BF16, kind="ExternalInput")
    w0p = nc.dram_tensor("w0p", [P, D], BF16, kind="ExternalInput")
    w1 = nc.dram_tensor("w1", [H1, H2], BF16, kind="ExternalInput")
    # w2g columns are (enc_w2[:, :4] @ e) for the 40 Fourier sign classes
    # (cos and sin rows share a column; biases differ by pi/2), col 0 = 0:
    # Sin(w2g^T h2 + biasg) IS the 81-row Fourier feature stack.
    w2g = nc.dram_tensor("w2g", [H2, 81], BF16, kind="ExternalInput")
    wfg = nc.dram_tensor("wfg", [81, H2], BF16, kind="ExternalInput")
    wd1 = nc.dram_tensor("wd1", [H2, H1], BF16, kind="ExternalInput")
    wd2 = nc.dram_tensor("wd2", [H1, D], BF16, kind="ExternalInput")
    bia = nc.dram_tensor("bia", [P, 21], F32, kind="ExternalInput")
    # fp16 output: |out| <= ~1e-3 so fp16 keeps ~2.4e-4 relative precision
    # while halving output DMA bytes; host converts back to f32.
    # outr row n*128+p, col k*512+c = out^T[k*128+p, n*512+c]: chunk-major
    # contiguous like xin, so each 4-block group is one contiguous 512KB
    # write DMA (4KB per partition row).
    outr = nc.dram_tensor("outr", [NCH * P, KD * CW], F16, kind="ExternalOutput")

    Relu = mybir.ActivationFunctionType.Relu
    Sin = mybir.ActivationFunctionType.Sin

    with tile.TileContext(nc) as tc:
        with (
            tc.tile_pool(name="const", bufs=1) as cpool,
            tc.tile_pool(name="xin", bufs=4) as xpool,
            tc.tile_pool(name="work", bufs=2) as wpool,
            tc.tile_pool(name="stage", bufs=6) as spool,
            tc.tile_pool(name="ps1p", bufs=2, space="PSUM") as ps1pool,
            tc.tile_pool(name="psmid", bufs=2, space="PSUM") as psmpool,
            tc.tile_pool(name="psout", bufs=2, space="PSUM") as psopool,
        ):
            # ---- resident constant tiles (DMAs interleaved below) ----
            w0p_sb = cpool.tile([P, D], BF16, name="w0p_sb", tag="w0p_sb")
            w1_sb = cpool.tile([H1, H2], BF16, name="w1_sb", tag="w1_sb")
            w2g_sb = cpool.tile([H2, 81], BF16, name="w2g_sb", tag="w2g_sb")
            wfg_sb = cpool.tile([81, H2], BF16, name="wfg_sb", tag="wfg_sb")
            wd1_sb = cpool.tile([H2, H1], BF16, name="wd1_sb", tag="wd1_sb")
            wd2_sb = cpool.tile([H1, D], BF16, name="wd2_sb", tag="wd2_sb")
            bia_sb = cpool.tile([P, 21], F32, name="bia_sb", tag="bia_sb")

            # ---- x in: raw f32 chunk-major [128, 512] tiles, all on the
            # sync HWDGE ring (sync has no compute duties, so ring-full
            # stalls are harmless). Chunk 0's tiles are interleaved with
            # w0p's 16 [128,128] column blocks so L1(0) step k has both its
            # weight block and x tile as early as possible. The small
            # weights go on the scalar HWDGE ring up front: 7 triggers that
            # drain in ~2us, long before the first eviction needs the
            # scalar engine. ----
            nc.scalar.dma_start(out=bia_sb[:], in_=bia[:])
            nc.scalar.dma_start(out=w1_sb[:], in_=w1[:])
            nc.scalar.dma_start(out=w2g_sb[:], in_=w2g[:])
            nc.scalar.dma_start(out=wfg_sb[:], in_=wfg[:])
            nc.scalar.dma_start(out=wd1_sb[:], in_=wd1[:])
            nc.scalar.dma_start(out=wd2_sb[:], in_=wd2[:])

            nc.sync.dma_start(out=w0p_sb[:], in_=w0p[:])
            xts = {}
            for n in range(NCH):
                for q in range(2):
                    t = xpool.tile([P, 8 * CW], BF16, name=f"x_{n}_{q}", tag=f"xt{q}")
                    nc.sync.dma_start(
                        out=t[:],
                        in_=xin[n * P : (n + 1) * P, q * 8 * CW : (q + 1) * 8 * CW],
                    )
                    xts[(n, q)] = t

            def x_slice(n, k):
                return xts[(n, k // 8)][:, (k % 8) * CW : (k % 8 + 1) * CW]

            state = {}

            def emit_l1(n):
                """16 accumulating f32r matmuls + relu eviction for chunk n."""
                ps1 = ps1pool.tile([P, CW], F32, name=f"ps1_{n}", tag="ps1")
                h1 = wpool.tile([P, CW], BF16, name=f"h1_{n}", tag="h1")
                state[n] = h1
                steps = []
                for k in range(KD):
                    def mm(k=k, ps1=ps1, n=n):
                        nc.tensor.matmul(
                            ps1[:],
                            w0p_sb[:, k * P : (k + 1) * P],
                            x_slice(n, k),
                            start=(k == 0),
                            stop=(k == KD - 1),
                        )
                    steps.append(mm)
                def ev(ps1=ps1, h1=h1):
                    nc.scalar.activation(h1[:], ps1[:], Relu, bias=bia_sb[:, 0:1])
                steps.append(ev)
                return steps

            def emit_mid(n):
                """Mid layers for batch chunk n (produces h4)."""
                h1 = state[n]
                steps = []

                ps2 = psmpool.tile([H2, CW], F32, name=f"ps2_{n}", tag="psm")
                h2 = wpool.tile([H2, CW], BF16, name=f"h2_{n}", tag="h2")
                steps.append(lambda: nc.tensor.matmul(ps2[:], w1_sb[:], h1[:], start=True, stop=True))
                steps.append(lambda: nc.scalar.activation(h2[:], ps2[:], Relu, bias=bia_sb[0:H2, 1:2]))

                ps3 = psmpool.tile([81, CW], F32, name=f"ps3_{n}", tag="psm")
                fst = wpool.tile([81, CW], BF16, name=f"fst_{n}", tag="fst")
                steps.append(lambda: nc.tensor.matmul(ps3[:], w2g_sb[:], h2[:], start=True, stop=True))
                steps.append(lambda: nc.scalar.activation(fst[:], ps3[:], Sin, bias=bia_sb[0:81, 2:3]))

                ps4 = psmpool.tile([H2, CW], F32, name=f"ps4_{n}", tag="psm")
                h3 = wpool.tile([H2, CW], BF16, name=f"h3_{n}", tag="h3")
                steps.append(lambda: nc.tensor.matmul(ps4[:], wfg_sb[:], fst[:], start=True, stop=True))
                steps.append(lambda: nc.scalar.activation(h3[:], ps4[:], Relu, bias=bia_sb[0:H2, 3:4]))

                ps5 = psmpool.tile([H1, CW], F32, name=f"ps5_{n}", tag="psm")
                h4 = wpool.tile([H1, CW], BF16, name=f"h4_{n}", tag="h4")
                steps.append(lambda: nc.tensor.matmul(ps5[:], wd1_sb[:], h3[:], start=True, stop=True))
                steps.append(lambda: nc.scalar.activation(h4[:], ps5[:], Relu, bias=bia_sb[0:H1, 4:5]))
                state[("h4", n)] = h4
                return steps

            def emit_dec(n):
                """Decoder head + out-DMAs for batch chunk n. Evictions
                stage 4 column blocks into one [128, 2048] f16 tile, then a
                single contiguous 512KB DMA writes the group; chunk 0's
                writes go on gpsimd (sync is still streaming x), later
                chunks on the by-then-idle sync ring."""
                h4 = state[("h4", n)]
                steps = []
                for g in range(4):
                    ost4 = spool.tile([P, 4 * CW], F16, name=f"ost_{n}_{g}", tag="ost")
                    for h in range(2):
                        # pair two column blocks in one 2-bank PSUM tile so a
                        # single 1024-wide eviction drains both
                        ps6 = psopool.tile([P, 2 * CW], F32, name=f"ps6_{n}_{g}_{h}", tag="pso")
                        for j in range(2):
                            mg = 4 * g + 2 * h + j
                            def mm6(ps6=ps6, j=j, mg=mg, h4=h4):
                                nc.tensor.matmul(
                                    ps6[:, j * CW : (j + 1) * CW],
                                    wd2_sb[:, mg * P : (mg + 1) * P],
                                    h4[:],
                                    start=True,
                                    stop=True,
                                )
                            steps.append(mm6)
                        dst = ost4[:, 2 * h * CW : 2 * (h + 1) * CW]
                        use_dve = (g + h + n) % 2 == 0
                        if dec2_bias_zero:
                            if use_dve:
                                def ev6(dst=dst, ps6=ps6):
                                    nc.vector.tensor_copy(dst, ps6[:])
                            else:
                                def ev6(dst=dst, ps6=ps6):
                                    nc.scalar.copy(dst, ps6[:])
                            steps.append(ev6)
                        else:
                            mg0 = 4 * g + 2 * h
                            if use_dve:
                                def ev6(dst=dst, ps6=ps6, mg0=mg0):
                                    for j in range(2):
                                        nc.vector.tensor_scalar_add(
                                            dst[:, j * CW : (j + 1) * CW],
                                            ps6[:, j * CW : (j + 1) * CW],
                                            bia_sb[:, 5 + mg0 + j : 6 + mg0 + j],
                                        )
                            else:
                                def ev6(dst=dst, ps6=ps6, mg0=mg0):
                                    for j in range(2):
                                        nc.scalar.add(
                                            dst[:, j * CW : (j + 1) * CW],
                                            ps6[:, j * CW : (j + 1) * CW],
                                            bia_sb[:, 5 + mg0 + j : 6 + mg0 + j],
                                        )
                            steps.append(ev6)
                    def dma6(ost4=ost4, g=g, n=n):
                        eng = nc.gpsimd if n <= 1 else nc.sync
                        eng.dma_start(
                            out=outr[n * P : (n + 1) * P, g * 4 * CW : (g + 1) * 4 * CW],
                            in_=ost4[:],
                        )
                    steps.append(dma6)
                return steps

            def ratio_merge(*streams):
                streams = [s for s in streams if s]
                out = []
                idx = [0] * len(streams)
                total = sum(len(s) for s in streams)
                for _ in range(total):
                    best, bestv = None, None
                    for si, s in enumerate(streams):
                        if idx[si] >= len(s):
                            continue
                        v = idx[si] / len(s)
                        if bestv is None or v < bestv:
                            best, bestv = si, v
                    out.append(streams[best][idx[best]])
                    idx[best] += 1
                return out

            # preload the Sin ACT table set while the Scalar engine is idle
            dsin = cpool.tile([1, 8], F32, name="dsin", tag="dsin")
            nc.vector.memset(dsin[:], 0.0)
            nc.scalar.activation(dsin[:, 4:8], dsin[:, 0:4], Sin)

            # PE warm-up: short narrow dummy matmuls warm the HAM clock gate
            # while w0p + the first x piece stream in, without delaying L1(0)
            # by more than ~2us (128-wide MMs are ~160ns even cold).
            wtile = cpool.tile([P, CW], BF16, name="wtile", tag="wtile")
            nc.vector.memset(wtile[:], 0.0)
            if warmup:
                wps = psopool.tile([P, 128], F32, name="wps", tag="pso")
                for i in range(warmup):
                    nc.tensor.matmul(
                        wps[:, 0:128], wtile[:, 0:128], wtile[:, 0:128],
                        start=(i == 0), stop=(i == warmup - 1),
                    )

            # software pipeline: L1(n) interleaved with mid(n-1) followed by
            # dec(n-1) (one-chunk latency so writes start early); small dummy
            # bursts in n=1..2 bridge the x-starvation gaps and keep HAM warm
            for n in range(NCH + 1):
                if n in (1, 2) and warmup2:
                    wpsn = psopool.tile([P, CW], F32, name=f"wps{n}", tag="pso")
                    for i in range(warmup2):
                        nc.tensor.matmul(
                            wpsn[:], wtile[:, 0:128], wtile[:], start=(i == 0),
                            stop=(i == warmup2 - 1),
                        )
                a = emit_l1(n) if n < NCH else []
                b = emit_mid(n - 1) + emit_dec(n - 1) if 1 <= n <= NCH else []
                for step in ratio_merge(a, b):
                    step()

    nc.compile()
    _PROGRAM_CACHE[key] = nc
    return nc


# =====================================================================
# Host wrapper: shard, run, gather
# =====================================================================


def make_in_maps(
    x, enc_w0, enc_b0, enc_w1, enc_b1, enc_w2, enc_b2, qw,
    dec_w0, dec_b0, dec_w1, dec_b1, dec_w2, dec_b2,
):
    f32 = np.float32
    # circuit collapsed to the Fourier feature basis, folded into dec_w0
    w2g_h, biasg, wfg_h = _fourier_mid(qw, enc_w2, enc_b2, dec_w0)

    # enc_w0 repacked so SBUF col-block k holds rows k*128..(k+1)*128
    w0p = (
        np.asarray(enc_w0, f32).reshape(KD, P, H1).transpose(1, 0, 2).reshape(P, D)
    )
    w0p = np.ascontiguousarray(w0p)

    bia = np.zeros((P, 21), dtype=f32)
    bia[:, 0] = enc_b0
    bia[:H2, 1] = enc_b1
    bia[:81, 2] = biasg  # Fourier stack phases (incl. enc_b2 contribution)
    bia[:H2, 3] = dec_b0
    bia[:H1, 4] = dec_b1
    bia[:, 5 : 5 + KD] = np.asarray(dec_b2, f32).reshape(KD, P).T

    bf16 = ml_dtypes.bfloat16
    common = {
        "w0p": w0p.astype(bf16),
        "w1": np.ascontiguousarray(np.asarray(enc_w1, f32)).astype(bf16),
        "w2g": np.ascontiguousarray(w2g_h).astype(bf16),
        "wfg": np.ascontiguousarray(wfg_h).astype(bf16),
        "wd1": np.ascontiguousarray(np.asarray(dec_w1, f32)).astype(bf16),
        "wd2": np.ascontiguousarray(np.asarray(dec_w2, f32)).astype(bf16),
        "bia": bia,
    }

    # repack x per core: xin[n*128+p, k*512+c] = x[c0 + n*512 + c, k*128+p]
    # cast to bf16 on host (same rounding the device DMA cast would do)
    xr = np.ascontiguousarray(
        np.asarray(x, f32)
        .reshape(NCORES, NCH, CW, KD, P)
        .transpose(0, 1, 4, 3, 2)
    ).astype(bf16).reshape(NCORES, NCH * P, KD * CW)
    in_maps = []
    for c in range(NCORES):
        m = dict(common)
        m["xin"] = xr[c]
        in_maps.append(m)
    return in_maps


def gather_output(results):
    # outr[n*128+p, k*512+c] = out[c0 + n*512 + c, k*128 + p]
    arr = np.stack([results[c]["outr"] for c in range(NCORES)])
    arr = arr.reshape(NCORES, NCH, P, KD, CW).transpose(0, 1, 4, 3, 2)
    return np.ascontiguousarray(arr).reshape(B, D).astype(np.float32)


def kernel(**inputs):
    nc = _build_program(
        dec2_bias_zero=not np.any(np.asarray(inputs["dec_b2"], np.float32))
    )
    in_maps = make_in_maps(**inputs)
    res = run_bass_kernel_spmd(nc, in_maps, core_ids=list(range(NCORES)))
    return gather_output(res.results)


if __name__ == "__main__":
    rng = np.random.default_rng(0)
    demo = {
        "x": rng.normal(size=(B, D)).astype(np.float32),
        "enc_w0": rng.normal(size=(D, H1)).astype(np.float32) * 0.02,
        "enc_b0": np.zeros(H1, np.float32),
        "enc_w1": rng.normal(size=(H1, H2)).astype(np.float32) * 0.02,
        "enc_b1": np.zeros(H2, np.float32),
        "enc_w2": rng.normal(size=(H2, L)).astype(np.float32) * 0.02,
        "enc_b2": np.zeros(L, np.float32),
        "qw": rng.normal(size=(NL, NQ, 3)).astype(np.float32),
        "dec_w0": rng.normal(size=(NQ, H2)).astype(np.float32) * 0.02,
        "dec_b0": np.zeros(H2, np.float32),
        "dec_w1": rng.normal(size=(H2, H1)).astype(np.float32) * 0.02,
        "dec_w2": rng.normal(size=(H1, D)).astype(np.float32) * 0.02,
        "dec_b1": np.zeros(H1, np.float32),
        "dec_b2": np.zeros(D, np.float32),
    }
    out = kernel(**demo)
    print("kernel ran, out shape:", out.shape, "finite:", np.isfinite(out).all())


# revision 3
# speedup vs baseline: 1.0073x; 1.0073x over previous
"""Trainium2 Bass kernel for nn_Autoencoder__gen204 (8-core data parallel).

Math: encoder 2048->128->64 MLP; the 4-qubit circuit on latent[:, :4] is
collapsed ON HOST into a Fourier feature layer — refined = Wnew^T G(t) with
G(t) = [1, cos(e.t), sin(e.t)] over the 40 sign classes e in {-1,0,1}^4 —
so the device computes it as ONE matmul (angle rows) + ONE Sin activation
+ one folded matmul (Wnew @ dec_w0); then the decoder 64->128->2048.

DMA/engine plan (measured on HW):
- x is cast to bf16 ON HOST and repacked chunk-major-contiguous: the read
  stream is 8.4MB (not 16.8 f32) and each half-chunk piece is one fully
  contiguous ~1MB DMA (8KB/partition row) -> sustains ~415 GB/s/core.
- x rides the sync HWDGE ring: the sync engine has no compute duties, so
  its ~0.7us dma_start triggers never block compute (a scalar-ring x
  stream deadlocks evictions behind the triggers; gpsimd delays writes).
- all matmuls bf16 (f32r measured ~1.7x slower per MM than the cost
  model claims: fp32_mode=HIGH, 628-795ns vs 379ns at equal clock).
- small weights ride the scalar ring up front; wd2 rides sync BEHIND
  chunks 0-1 (needed only at dec(0)).
- output f16, staged 2048-wide and written chunk-major-contiguous:
  chunks 0-1 on gpsimd (SWDGE) while sync still streams x, chunks 2-3 on
  sync; the final chunk splits each write across both rings.
- software pipeline L1(n) || mid(n-1)+dec(n-1), 512-col chunks; short
  128-wide warmup + small dummy-MM bursts at n=1,2 hold the HAM clock
  gate at 8/8 through the x-starved ramp.
"""

import ml_dtypes
import numpy as np

import concourse.bass as bass
import concourse.mybir as mybir
import concourse.tile as tile
from concourse import bacc
from concourse.bass_utils import run_bass_kernel_spmd

# ----- problem constants (hardcoded per contract) -----
B, D, H1, H2, L = 16384, 2048, 128, 64, 32
NQ, NL = 4, 3
NCORES = 8
BL = B // NCORES  # 2048 batch per core
P = 128
KD = D // P  # 16 k-chunks for the D contraction
NCH = 4  # batch chunks per core
CW = BL // NCH  # 512 cols per chunk
F32 = mybir.dt.float32
F32R = mybir.dt.float32r
BF16 = mybir.dt.bfloat16
F16 = mybir.dt.float16

# =====================================================================
# Host-side quantum-circuit collapse: qw -> W81 (81, 4)
# =====================================================================

_I2 = np.eye(2, dtype=np.complex128)
_SY = np.array([[0, -1j], [1j, 0]], dtype=np.complex128)
_SZ = np.array([[1, 0], [0, -1]], dtype=np.complex128)
_CNOT4 = np.array(
    [[1, 0, 0, 0], [0, 1, 0, 0], [0, 0, 0, 1], [0, 0, 1, 0]], dtype=np.complex128
).reshape(2, 2, 2, 2)
_bits = (np.arange(2**NQ)[:, None] >> np.arange(NQ - 1, -1, -1)) & 1
_Z_SIGNS = (1 - 2 * _bits).astype(np.float64)  # (16, 4)


def _rot_mat(phi, theta, omega):
    ez = np.exp(-0.5j * phi)
    rz1 = np.array([[ez, 0], [0, np.conj(ez)]], dtype=np.complex128)
    c, s = np.cos(theta / 2), np.sin(theta / 2)
    ry = np.array([[c, -s], [s, c]], dtype=np.complex128)
    eo = np.exp(-0.5j * omega)
    rz2 = np.array([[eo, 0], [0, np.conj(eo)]], dtype=np.complex128)
    return rz2 @ ry @ rz1


def _apply1(state, U, wire):
    state = np.tensordot(U, state, axes=[[1], [wire]])
    return np.moveaxis(state, 0, wire)


def _apply_cnot(state, c, t):
    state = np.tensordot(_CNOT4, state, axes=[[2, 3], [c, t]])
    return np.moveaxis(state, [0, 1], [c, t])


def _w81_from_qw(qw):
    qw = np.asarray(qw, dtype=np.float64)
    V = np.eye(16, dtype=np.complex128).reshape(2, 2, 2, 2, 16)
    for layer in range(NL):
        for q in range(NQ):
            V = _apply1(V, _rot_mat(*qw[layer, q]), q)
        for q in range(NQ - 1):
            V = _apply_cnot(V, q, q + 1)
    V = V.reshape(16, 16)
    paulis = [_I2, _SY, _SZ]  # digit 0 -> I(1), 1 -> Y(sin), 2 -> Z(cos)
    W = np.zeros((81, NQ), dtype=np.float64)
    for q in range(NQ):
        O = V.conj().T @ (_Z_SIGNS[:, q][:, None] * V)
        for k in range(81):
            d = [(k // 27) % 3, (k // 9) % 3, (k // 3) % 3, k % 3]
            Pm = np.kron(
                np.kron(paulis[d[0]], paulis[d[1]]),
                np.kron(paulis[d[2]], paulis[d[3]]),
            )
            alpha = np.trace(O @ Pm) / 16.0
            n_y = sum(1 for x in d if x == 1)
            W[k, q] = alpha.real * ((-1) ** n_y)  # t(Y) = -sin; feature is +sin
    return W


def _sign_classes():
    """(40, 4) int: e in {-1,0,1}^4 \\ {0} mod global sign (first nonzero +1)."""
    E = []
    for a in (-1, 0, 1):
        for b in (-1, 0, 1):
            for c in (-1, 0, 1):
                for d in (-1, 0, 1):
                    e = (a, b, c, d)
                    if e == (0, 0, 0, 0):
                        continue
                    if next(x for x in e if x != 0) < 0:
                        continue
                    E.append(e)
    return np.array(E, dtype=np.float64)


def _fourier_mid(qw, enc_w2, enc_b2, dec_w0):
    """Rewrite refined = W81^T phi(t) in the Fourier basis
    G(t) = [1, cos(e.t), sin(e.t)]_e so the device computes the 81 features
    with ONE matmul (args) + ONE Sin activation instead of selector matmuls
    and elementwise products.  Returns (w2g [64,81], biasg [81], wfg [81,64]).
    """
    w81 = _w81_from_qw(np.asarray(qw, np.float64))  # (81, NQ)
    E = _sign_classes()  # (40, 4)

    def phi_row(t):
        fs = np.stack([np.ones(NQ), np.sin(t), np.cos(t)])
        out = np.empty(81)
        for k in range(81):
            d = [(k // 27) % 3, (k // 9) % 3, (k // 3) % 3, k % 3]
            out[k] = np.prod([fs[d[q], q] for q in range(NQ)])
        return out

    def g_row(t):
        u = E @ t
        g = np.empty(81)
        g[0] = 1.0
        g[1::2] = np.cos(u)
        g[2::2] = np.sin(u)
        return g

    rng = np.random.default_rng(12345)
    T = rng.normal(size=(400, NQ)) * 2.0
    GT = np.stack([g_row(t) for t in T])
    PT = np.stack([phi_row(t) for t in T])
    Wnew, *_ = np.linalg.lstsq(GT, PT @ w81, rcond=None)  # (81, NQ)

    w2q = np.asarray(enc_w2, np.float64)[:, :NQ]  # (64, 4)
    b2q = np.asarray(enc_b2, np.float64)[:NQ]
    cols = w2q @ E.T  # (64, 40)
    w2g = np.zeros((H2, 81))
    w2g[:, 1::2] = cols
    w2g[:, 2::2] = cols
    ub = E @ b2q  # (40,)
    biasg = np.zeros(81)
    biasg[0] = np.pi / 2
    biasg[1::2] = ub + np.pi / 2  # sin(u + pi/2) = cos(u)
    biasg[2::2] = ub
    wfg = Wnew @ np.asarray(dec_w0, np.float64)  # (81, 64)
    return (
        w2g.astype(np.float32),
        biasg.astype(np.float32),
        wfg.astype(np.float32),
    )


# =====================================================================
# Bass program (one core; identical across the 8 cores)
# =====================================================================

_PROGRAM_CACHE = {}


def _build_program(debug=False, dec2_bias_zero=False, warmup=12, warmup2=6):
    key = (debug, dec2_bias_zero, warmup, warmup2)
    if key in _PROGRAM_CACHE:
        return _PROGRAM_CACHE[key]

    nc = bacc.Bacc("TRN2", target_bir_lowering=False, debug=debug)

    # xin row n*128+p, col k*512+c  =  x^T[k*128+p, n*512+c]: host-repacked
    # bf16 so each half-chunk piece [128, 4096] is one fully contiguous
    # ~1MB DMA (8KB per partition row) -> few triggers, max HBM efficiency.
    xin = nc.dram_tensor("xin", [NCH * P, KD * CW], BF16, kind="ExternalInput")
    w0p = nc.dram_tensor("w0p", [P, D], BF16, kind="ExternalInput")
    w1 = nc.dram_tensor("w1", [H1, H2], BF16, kind="ExternalInput")
    # w2g columns are (enc_w2[:, :4] @ e) for the 40 Fourier sign classes
    # (cos and sin rows share a column; biases differ by pi/2), col 0 = 0:
    # Sin(w2g^T h2 + biasg) IS the 81-row Fourier feature stack.
    w2g = nc.dram_tensor("w2g", [H2, 81], BF16, kind="ExternalInput")
    wfg = nc.dram_tensor("wfg", [81, H2], BF16, kind="ExternalInput")
    wd1 = nc.dram_tensor("wd1", [H2, H1], BF16, kind="ExternalInput")
    wd2 = nc.dram_tensor("wd2", [H1, D], BF16, kind="ExternalInput")
    bia = nc.dram_tensor("bia", [P, 21], F32, kind="ExternalInput")
    # fp16 output: |out| <= ~1e-3 so fp16 keeps ~2.4e-4 relative precision
    # while halving output DMA bytes; host converts back to f32.
    # outr row n*128+p, col k*512+c = out^T[k*128+p, n*512+c]: chunk-major
    # contiguous like xin, so each 4-block group is one contiguous 512KB
    # write DMA (4KB per partition row).
    outr = nc.dram_tensor("outr", [NCH * P, KD * CW], F16, kind="ExternalOutput")

    Relu = mybir.ActivationFunctionType.Relu
    Sin = mybir.ActivationFunctionType.Sin

    with tile.TileContext(nc) as tc:
        with (
            tc.tile_pool(name="const", bufs=1) as cpool,
            tc.tile_pool(name="xin", bufs=4) as xpool,
            tc.tile_pool(name="work", bufs=2) as wpool,
            tc.tile_pool(name="stage", bufs=6) as spool,
            tc.tile_pool(name="ps1p", bufs=2, space="PSUM") as ps1pool,
            tc.tile_pool(name="psmid", bufs=2, space="PSUM") as psmpool,
            tc.tile_pool(name="psout", bufs=2, space="PSUM") as psopool,
        ):
            # ---- resident constant tiles (DMAs interleaved below) ----
            w0p_sb = cpool.tile([P, D], BF16, name="w0p_sb", tag="w0p_sb")
            w1_sb = cpool.tile([H1, H2], BF16, name="w1_sb", tag="w1_sb")
            w2g_sb = cpool.tile([H2, 81], BF16, name="w2g_sb", tag="w2g_sb")
            wfg_sb = cpool.tile([81, H2], BF16, name="wfg_sb", tag="wfg_sb")
            wd1_sb = cpool.tile([H2, H1], BF16, name="wd1_sb", tag="wd1_sb")
            wd2_sb = cpool.tile([H1, D], BF16, name="wd2_sb", tag="wd2_sb")
            bia_sb = cpool.tile([P, 21], F32, name="bia_sb", tag="bia_sb")

            # ---- x in: raw f32 chunk-major [128, 512] tiles, all on the
            # sync HWDGE ring (sync has no compute duties, so ring-full
            # stalls are harmless). Chunk 0's tiles are interleaved with
            # w0p's 16 [128,128] column blocks so L1(0) step k has both its
            # weight block and x tile as early as possible. The small
            # weights go on the scalar HWDGE ring up front: 7 triggers that
            # drain in ~2us, long before the first eviction needs the
            # scalar engine. ----
            nc.scalar.dma_start(out=bia_sb[:], in_=bia[:])
            nc.scalar.dma_start(out=w1_sb[:], in_=w1[:])
            nc.scalar.dma_start(out=w2g_sb[:], in_=w2g[:])
            nc.scalar.dma_start(out=wfg_sb[:], in_=wfg[:])
            nc.scalar.dma_start(out=wd1_sb[:], in_=wd1[:])

            nc.sync.dma_start(out=w0p_sb[:], in_=w0p[:])
            xts = {}
            for n in range(NCH):
                for q in range(2):
                    t = xpool.tile([P, 8 * CW], BF16, name=f"x_{n}_{q}", tag=f"xt{q}")
                    nc.sync.dma_start(
                        out=t[:],
                        in_=xin[n * P : (n + 1) * P, q * 8 * CW : (q + 1) * 8 * CW],
                    )
                    xts[(n, q)] = t
                if n == 1:
                    # wd2 is only needed at dec(0) (~18us): queue it on sync
                    # BEHIND chunks 0-1 so it delays late chunks, not chunk 0
                    nc.sync.dma_start(out=wd2_sb[:], in_=wd2[:])

            def x_slice(n, k):
                return xts[(n, k // 8)][:, (k % 8) * CW : (k % 8 + 1) * CW]

            state = {}

            def emit_l1(n):
                """16 accumulating f32r matmuls + relu eviction for chunk n."""
                ps1 = ps1pool.tile([P, CW], F32, name=f"ps1_{n}", tag="ps1")
                h1 = wpool.tile([P, CW], BF16, name=f"h1_{n}", tag="h1")
                state[n] = h1
                steps = []
                for k in range(KD):
                    def mm(k=k, ps1=ps1, n=n):
                        nc.tensor.matmul(
                            ps1[:],
                            w0p_sb[:, k * P : (k + 1) * P],
                            x_slice(n, k),
                            start=(k == 0),
                            stop=(k == KD - 1),
                        )
                    steps.append(mm)
                def ev(ps1=ps1, h1=h1):
                    nc.scalar.activation(h1[:], ps1[:], Relu, bias=bia_sb[:, 0:1])
                steps.append(ev)
                return steps

            def emit_mid(n):
                """Mid layers for batch chunk n (produces h4)."""
                h1 = state[n]
                steps = []

                ps2 = psmpool.tile([H2, CW], F32, name=f"ps2_{n}", tag="psm")
                h2 = wpool.tile([H2, CW], BF16, name=f"h2_{n}", tag="h2")
                steps.append(lambda: nc.tensor.matmul(ps2[:], w1_sb[:], h1[:], start=True, stop=True))
                steps.append(lambda: nc.scalar.activation(h2[:], ps2[:], Relu, bias=bia_sb[0:H2, 1:2]))

                ps3 = psmpool.tile([81, CW], F32, name=f"ps3_{n}", tag="psm")
                fst = wpool.tile([81, CW], BF16, name=f"fst_{n}", tag="fst")
                steps.append(lambda: nc.tensor.matmul(ps3[:], w2g_sb[:], h2[:], start=True, stop=True))
                steps.append(lambda: nc.scalar.activation(fst[:], ps3[:], Sin, bias=bia_sb[0:81, 2:3]))

                ps4 = psmpool.tile([H2, CW], F32, name=f"ps4_{n}", tag="psm")
                h3 = wpool.tile([H2, CW], BF16, name=f"h3_{n}", tag="h3")
                steps.append(lambda: nc.tensor.matmul(ps4[:], wfg_sb[:], fst[:], start=True, stop=True))
                steps.append(lambda: nc.scalar.activation(h3[:], ps4[:], Relu, bias=bia_sb[0:H2, 3:4]))

                ps5 = psmpool.tile([H1, CW], F32, name=f"ps5_{n}", tag="psm")
                h4 = wpool.tile([H1, CW], BF16, name=f"h4_{n}", tag="h4")
                steps.append(lambda: nc.tensor.matmul(ps5[:], wd1_sb[:], h3[:], start=True, stop=True))
                steps.append(lambda: nc.scalar.activation(h4[:], ps5[:], Relu, bias=bia_sb[0:H1, 4:5]))
                state[("h4", n)] = h4
                return steps

            def emit_dec(n):
                """Decoder head + out-DMAs for batch chunk n. Evictions
                stage 4 column blocks into one [128, 2048] f16 tile, then a
                single contiguous 512KB DMA writes the group; chunk 0's
                writes go on gpsimd (sync is still streaming x), later
                chunks on the by-then-idle sync ring."""
                h4 = state[("h4", n)]
                steps = []
                for g in range(4):
                    ost4 = spool.tile([P, 4 * CW], F16, name=f"ost_{n}_{g}", tag="ost")
                    for h in range(2):
                        # pair two column blocks in one 2-bank PSUM tile so a
                        # single 1024-wide eviction drains both
                        ps6 = psopool.tile([P, 2 * CW], F32, name=f"ps6_{n}_{g}_{h}", tag="pso")
                        for j in range(2):
                            mg = 4 * g + 2 * h + j
                            def mm6(ps6=ps6, j=j, mg=mg, h4=h4):
                                nc.tensor.matmul(
                                    ps6[:, j * CW : (j + 1) * CW],
                                    wd2_sb[:, mg * P : (mg + 1) * P],
                                    h4[:],
                                    start=True,
                                    stop=True,
                                )
                            steps.append(mm6)
                        dst = ost4[:, 2 * h * CW : 2 * (h + 1) * CW]
                        use_dve = (g + h + n) % 2 == 0
                        if dec2_bias_zero:
                            if use_dve:
                                def ev6(dst=dst, ps6=ps6):
                                    nc.vector.tensor_copy(dst, ps6[:])
                            else:
                                def ev6(dst=dst, ps6=ps6):
                                    nc.scalar.copy(dst, ps6[:])
                            steps.append(ev6)
                        else:
                            mg0 = 4 * g + 2 * h
                            if use_dve:
                                def ev6(dst=dst, ps6=ps6, mg0=mg0):
                                    for j in range(2):
                                        nc.vector.tensor_scalar_add(
                                            dst[:, j * CW : (j + 1) * CW],
                                            ps6[:, j * CW : (j + 1) * CW],
                                            bia_sb[:, 5 + mg0 + j : 6 + mg0 + j],
                                        )
                            else:
                                def ev6(dst=dst, ps6=ps6, mg0=mg0):
                                    for j in range(2):
                                        nc.scalar.add(
                                            dst[:, j * CW : (j + 1) * CW],
                                            ps6[:, j * CW : (j + 1) * CW],
                                            bia_sb[:, 5 + mg0 + j : 6 + mg0 + j],
                                        )
                            steps.append(ev6)
                    def dma6(ost4=ost4, g=g, n=n):
                        if n == NCH - 1:
                            # final chunk: split each write across both rings
                            # so the tail drains two-wide
                            for h2_, eng in ((0, nc.sync), (1, nc.gpsimd)):
                                eng.dma_start(
                                    out=outr[
                                        n * P : (n + 1) * P,
                                        (2 * g + h2_) * 2 * CW : (2 * g + h2_ + 1) * 2 * CW,
                                    ],
                                    in_=ost4[:, h2_ * 2 * CW : (h2_ + 1) * 2 * CW],
                                )
                        else:
                            eng = nc.gpsimd if n <= 1 else nc.sync
                            eng.dma_start(
                                out=outr[n * P : (n + 1) * P, g * 4 * CW : (g + 1) * 4 * CW],
                                in_=ost4[:],
                            )
                    steps.append(dma6)
                return steps

            def ratio_merge(*streams):
                streams = [s for s in streams if s]
                out = []
                idx = [0] * len(streams)
                total = sum(len(s) for s in streams)
                for _ in range(total):
                    best, bestv = None, None
                    for si, s in enumerate(streams):
                        if idx[si] >= len(s):
                            continue
                        v = idx[si] / len(s)
                        if bestv is None or v < bestv:
                            best, bestv = si, v
                    out.append(streams[best][idx[best]])
                    idx[best] += 1
                return out

            # preload the Sin ACT table set while the Scalar engine is idle
            dsin = cpool.tile([1, 8], F32, name="dsin", tag="dsin")
            nc.vector.memset(dsin[:], 0.0)
            nc.scalar.activation(dsin[:, 4:8], dsin[:, 0:4], Sin)

            # PE warm-up: short narrow dummy matmuls warm the HAM clock gate
            # while w0p + the first x piece stream in, without delaying L1(0)
            # by more than ~2us (128-wide MMs are ~160ns even cold).
            wtile = cpool.tile([P, CW], BF16, name="wtile", tag="wtile")
            nc.vector.memset(wtile[:], 0.0)
            if warmup:
                wps = psopool.tile([P, 128], F32, name="wps", tag="pso")
                for i in range(warmup):
                    nc.tensor.matmul(
                        wps[:], wtile[:, 0:128], wtile[:, 0:128],
                        start=(i == 0), stop=(i == warmup - 1),
                    )

            # software pipeline: L1(n) interleaved with mid(n-1) followed by
            # dec(n-1) (one-chunk latency so writes start early); small dummy
            # bursts in n=1..2 bridge the x-starvation gaps and keep HAM warm
            for n in range(NCH + 1):
                if n in (1, 2) and warmup2:
                    wpsn = psopool.tile([P, CW], F32, name=f"wps{n}", tag="pso")
                    for i in range(warmup2):
                        nc.tensor.matmul(
                            wpsn[:], wtile[:, 0:128], wtile[:], start=(i == 0),
                            stop=(i == warmup2 - 1),
                        )
                a = emit_l1(n) if n < NCH else []
                b = emit_mid(n - 1) + emit_dec(n - 1) if 1 <= n <= NCH else []
                for step in ratio_merge(a, b):
                    step()

    nc.compile()
    _PROGRAM_CACHE[key] = nc
    return nc


# =====================================================================
# Host wrapper: shard, run, gather
# =====================================================================


def make_in_maps(
    x, enc_w0, enc_b0, enc_w1, enc_b1, enc_w2, enc_b2, qw,
    dec_w0, dec_b0, dec_w1, dec_b1, dec_w2, dec_b2,
):
    f32 = np.float32
    # circuit collapsed to the Fourier feature basis, folded into dec_w0
    w2g_h, biasg, wfg_h = _fourier_mid(qw, enc_w2, enc_b2, dec_w0)

    # enc_w0 repacked so SBUF col-block k holds rows k*128..(k+1)*128
    w0p = (
        np.asarray(enc_w0, f32).reshape(KD, P, H1).transpose(1, 0, 2).reshape(P, D)
    )
    w0p = np.ascontiguousarray(w0p)

    bia = np.zeros((P, 21), dtype=f32)
    bia[:, 0] = enc_b0
    bia[:H2, 1] = enc_b1
    bia[:81, 2] = biasg  # Fourier stack phases (incl. enc_b2 contribution)
    bia[:H2, 3] = dec_b0
    bia[:H1, 4] = dec_b1
    bia[:, 5 : 5 + KD] = np.asarray(dec_b2, f32).reshape(KD, P).T

    bf16 = ml_dtypes.bfloat16
    common = {
        "w0p": w0p.astype(bf16),
        "w1": np.ascontiguousarray(np.asarray(enc_w1, f32)).astype(bf16),
        "w2g": np.ascontiguousarray(w2g_h).astype(bf16),
        "wfg": np.ascontiguousarray(wfg_h).astype(bf16),
        "wd1": np.ascontiguousarray(np.asarray(dec_w1, f32)).astype(bf16),
        "wd2": np.ascontiguousarray(np.asarray(dec_w2, f32)).astype(bf16),
        "bia": bia,
    }

    # repack x per core: xin[n*128+p, k*512+c] = x[c0 + n*512 + c, k*128+p]
    # cast to bf16 on host (same rounding the device DMA cast would do)
    xr = np.ascontiguousarray(
        np.asarray(x, f32)
        .reshape(NCORES, NCH, CW, KD, P)
        .transpose(0, 1, 4, 3, 2)
    ).astype(bf16).reshape(NCORES, NCH * P, KD * CW)
    in_maps = []
    for c in range(NCORES):
        m = dict(common)
        m["xin"] = xr[c]
        in_maps.append(m)
    return in_maps


def gather_output(results):
    # outr[n*128+p, k*512+c] = out[c0 + n*512 + c, k*128 + p]
    arr = np.stack([results[c]["outr"] for c in range(NCORES)])
    arr = arr.reshape(NCORES, NCH, P, KD, CW).transpose(0, 1, 4, 3, 2)
    return np.ascontiguousarray(arr).reshape(B, D).astype(np.float32)


def kernel(**inputs):
    nc = _build_program(
        dec2_bias_zero=not np.any(np.asarray(inputs["dec_b2"], np.float32))
    )
    in_maps = make_in_maps(**inputs)
    res = run_bass_kernel_spmd(nc, in_maps, core_ids=list(range(NCORES)))
    return gather_output(res.results)


if __name__ == "__main__":
    rng = np.random.default_rng(0)
    demo = {
        "x": rng.normal(size=(B, D)).astype(np.float32),
        "enc_w0": rng.normal(size=(D, H1)).astype(np.float32) * 0.02,
        "enc_b0": np.zeros(H1, np.float32),
        "enc_w1": rng.normal(size=(H1, H2)).astype(np.float32) * 0.02,
        "enc_b1": np.zeros(H2, np.float32),
        "enc_w2": rng.normal(size=(H2, L)).astype(np.float32) * 0.02,
        "enc_b2": np.zeros(L, np.float32),
        "qw": rng.normal(size=(NL, NQ, 3)).astype(np.float32),
        "dec_w0": rng.normal(size=(NQ, H2)).astype(np.float32) * 0.02,
        "dec_b0": np.zeros(H2, np.float32),
        "dec_w1": rng.normal(size=(H2, H1)).astype(np.float32) * 0.02,
        "dec_w2": rng.normal(size=(H1, D)).astype(np.float32) * 0.02,
        "dec_b1": np.zeros(H1, np.float32),
        "dec_b2": np.zeros(D, np.float32),
    }
    out = kernel(**demo)
    print("kernel ran, out shape:", out.shape, "finite:", np.isfinite(out).all())


# revision 4
# speedup vs baseline: 1.0348x; 1.0273x over previous
"""Trainium2 Bass kernel for nn_Autoencoder__gen204 (8-core data parallel).

Math: encoder 2048->128->64 MLP; the 4-qubit circuit on latent[:, :4] is
collapsed ON HOST into a Fourier feature layer — refined = Wnew^T G(t) with
G(t) = [1, cos(e.t), sin(e.t)] over the 40 sign classes e in {-1,0,1}^4 —
so the device computes it as ONE matmul (angle rows) + ONE Sin activation
+ one folded matmul (Wnew @ dec_w0); then the decoder 64->128->2048.

DMA/engine plan (measured on HW):
- x is cast to bf16 ON HOST and repacked chunk-major-contiguous: the read
  stream is 8.4MB (not 16.8 f32) and each half-chunk piece is one fully
  contiguous ~1MB DMA (8KB/partition row) -> sustains ~415 GB/s/core.
- x rides the sync HWDGE ring: the sync engine has no compute duties, so
  its ~0.7us dma_start triggers never block compute (a scalar-ring x
  stream deadlocks evictions behind the triggers; gpsimd delays writes).
- all matmuls bf16 (f32r measured ~1.7x slower per MM than the cost
  model claims: fp32_mode=HIGH, 628-795ns vs 379ns at equal clock).
- small weights ride the scalar ring up front; wd2 rides sync BEHIND
  chunks 0-1 (needed only at dec(0)).
- output f16, staged 2048-wide and written chunk-major-contiguous:
  chunks 0-1 on gpsimd (SWDGE) while sync still streams x, chunks 2-3 on
  sync; the final chunk splits each write across both rings.
- software pipeline L1(n) || mid(n-1)+dec(n-1), 512-col chunks; short
  128-wide warmup + small dummy-MM bursts at n=1,2 hold the HAM clock
  gate at 8/8 through the x-starved ramp.
"""

import ml_dtypes
import numpy as np

import concourse.bass as bass
import concourse.mybir as mybir
import concourse.tile as tile
from concourse import bacc
from concourse.bass_utils import run_bass_kernel_spmd

# ----- problem constants (hardcoded per contract) -----
B, D, H1, H2, L = 16384, 2048, 128, 64, 32
NQ, NL = 4, 3
NCORES = 8
BL = B // NCORES  # 2048 batch per core
P = 128
KD = D // P  # 16 k-chunks for the D contraction
NCH = 4  # batch chunks per core
CW = BL // NCH  # 512 cols per chunk
F32 = mybir.dt.float32
F32R = mybir.dt.float32r
BF16 = mybir.dt.bfloat16
F16 = mybir.dt.float16

# =====================================================================
# Host-side quantum-circuit collapse: qw -> W81 (81, 4)
# =====================================================================

_I2 = np.eye(2, dtype=np.complex128)
_SY = np.array([[0, -1j], [1j, 0]], dtype=np.complex128)
_SZ = np.array([[1, 0], [0, -1]], dtype=np.complex128)
_CNOT4 = np.array(
    [[1, 0, 0, 0], [0, 1, 0, 0], [0, 0, 0, 1], [0, 0, 1, 0]], dtype=np.complex128
).reshape(2, 2, 2, 2)
_bits = (np.arange(2**NQ)[:, None] >> np.arange(NQ - 1, -1, -1)) & 1
_Z_SIGNS = (1 - 2 * _bits).astype(np.float64)  # (16, 4)


def _rot_mat(phi, theta, omega):
    ez = np.exp(-0.5j * phi)
    rz1 = np.array([[ez, 0], [0, np.conj(ez)]], dtype=np.complex128)
    c, s = np.cos(theta / 2), np.sin(theta / 2)
    ry = np.array([[c, -s], [s, c]], dtype=np.complex128)
    eo = np.exp(-0.5j * omega)
    rz2 = np.array([[eo, 0], [0, np.conj(eo)]], dtype=np.complex128)
    return rz2 @ ry @ rz1


def _apply1(state, U, wire):
    state = np.tensordot(U, state, axes=[[1], [wire]])
    return np.moveaxis(state, 0, wire)


def _apply_cnot(state, c, t):
    state = np.tensordot(_CNOT4, state, axes=[[2, 3], [c, t]])
    return np.moveaxis(state, [0, 1], [c, t])


def _w81_from_qw(qw):
    qw = np.asarray(qw, dtype=np.float64)
    V = np.eye(16, dtype=np.complex128).reshape(2, 2, 2, 2, 16)
    for layer in range(NL):
        for q in range(NQ):
            V = _apply1(V, _rot_mat(*qw[layer, q]), q)
        for q in range(NQ - 1):
            V = _apply_cnot(V, q, q + 1)
    V = V.reshape(16, 16)
    paulis = [_I2, _SY, _SZ]  # digit 0 -> I(1), 1 -> Y(sin), 2 -> Z(cos)
    W = np.zeros((81, NQ), dtype=np.float64)
    for q in range(NQ):
        O = V.conj().T @ (_Z_SIGNS[:, q][:, None] * V)
        for k in range(81):
            d = [(k // 27) % 3, (k // 9) % 3, (k // 3) % 3, k % 3]
            Pm = np.kron(
                np.kron(paulis[d[0]], paulis[d[1]]),
                np.kron(paulis[d[2]], paulis[d[3]]),
            )
            alpha = np.trace(O @ Pm) / 16.0
            n_y = sum(1 for x in d if x == 1)
            W[k, q] = alpha.real * ((-1) ** n_y)  # t(Y) = -sin; feature is +sin
    return W


def _sign_classes():
    """(40, 4) int: e in {-1,0,1}^4 \\ {0} mod global sign (first nonzero +1)."""
    E = []
    for a in (-1, 0, 1):
        for b in (-1, 0, 1):
            for c in (-1, 0, 1):
                for d in (-1, 0, 1):
                    e = (a, b, c, d)
                    if e == (0, 0, 0, 0):
                        continue
                    if next(x for x in e if x != 0) < 0:
                        continue
                    E.append(e)
    return np.array(E, dtype=np.float64)


def _fourier_mid(qw, enc_w2, enc_b2, dec_w0):
    """Rewrite refined = W81^T phi(t) in the Fourier basis
    G(t) = [1, cos(e.t), sin(e.t)]_e so the device computes the 81 features
    with ONE matmul (args) + ONE Sin activation instead of selector matmuls
    and elementwise products.  Returns (w2g [64,81], biasg [81], wfg [81,64]).
    """
    w81 = _w81_from_qw(np.asarray(qw, np.float64))  # (81, NQ)
    E = _sign_classes()  # (40, 4)

    def phi_row(t):
        fs = np.stack([np.ones(NQ), np.sin(t), np.cos(t)])
        out = np.empty(81)
        for k in range(81):
            d = [(k // 27) % 3, (k // 9) % 3, (k // 3) % 3, k % 3]
            out[k] = np.prod([fs[d[q], q] for q in range(NQ)])
        return out

    def g_row(t):
        u = E @ t
        g = np.empty(81)
        g[0] = 1.0
        g[1::2] = np.cos(u)
        g[2::2] = np.sin(u)
        return g

    rng = np.random.default_rng(12345)
    T = rng.normal(size=(400, NQ)) * 2.0
    GT = np.stack([g_row(t) for t in T])
    PT = np.stack([phi_row(t) for t in T])
    Wnew, *_ = np.linalg.lstsq(GT, PT @ w81, rcond=None)  # (81, NQ)

    w2q = np.asarray(enc_w2, np.float64)[:, :NQ]  # (64, 4)
    b2q = np.asarray(enc_b2, np.float64)[:NQ]
    cols = w2q @ E.T  # (64, 40)
    w2g = np.zeros((H2, 81))
    w2g[:, 1::2] = cols
    w2g[:, 2::2] = cols
    ub = E @ b2q  # (40,)
    biasg = np.zeros(81)
    biasg[0] = np.pi / 2
    biasg[1::2] = ub + np.pi / 2  # sin(u + pi/2) = cos(u)
    biasg[2::2] = ub
    wfg = Wnew @ np.asarray(dec_w0, np.float64)  # (81, 64)
    return (
        w2g.astype(np.float32),
        biasg.astype(np.float32),
        wfg.astype(np.float32),
    )


# =====================================================================
# Bass program (one core; identical across the 8 cores)
# =====================================================================

_PROGRAM_CACHE = {}


def _build_program(debug=False, dec2_bias_zero=False, warmup=12, warmup2=6):
    key = (debug, dec2_bias_zero, warmup, warmup2)
    if key in _PROGRAM_CACHE:
        return _PROGRAM_CACHE[key]

    nc = bacc.Bacc("TRN2", target_bir_lowering=False, debug=debug)

    # xin row n*128+p, col k*512+c  =  x^T[k*128+p, n*512+c]: host-repacked
    # bf16 so each half-chunk piece [128, 4096] is one fully contiguous
    # ~1MB DMA (8KB per partition row) -> few triggers, max HBM efficiency.
    xin = nc.dram_tensor("xin", [NCH * P, KD * CW], BF16, kind="ExternalInput")
    w0p = nc.dram_tensor("w0p", [P, D], BF16, kind="ExternalInput")
    w1 = nc.dram_tensor("w1", [H1, H2], BF16, kind="ExternalInput")
    # w2g columns are (enc_w2[:, :4] @ e) for the 40 Fourier sign classes
    # (cos and sin rows share a column; biases differ by pi/2), col 0 = 0:
    # Sin(w2g^T h2 + biasg) IS the 81-row Fourier feature stack.
    w2g = nc.dram_tensor("w2g", [H2, 81], BF16, kind="ExternalInput")
    wfg = nc.dram_tensor("wfg", [81, H2], BF16, kind="ExternalInput")
    wd1 = nc.dram_tensor("wd1", [H2, H1], BF16, kind="ExternalInput")
    wd2 = nc.dram_tensor("wd2", [H1, D], BF16, kind="ExternalInput")
    bia = nc.dram_tensor("bia", [P, 21], F32, kind="ExternalInput")
    # fp16 output: |out| <= ~1e-3 so fp16 keeps ~2.4e-4 relative precision
    # while halving output DMA bytes; host converts back to f32.
    # outr row n*128+p, col k*512+c = out^T[k*128+p, n*512+c]: chunk-major
    # contiguous like xin, so each 4-block group is one contiguous 512KB
    # write DMA (4KB per partition row).
    outr = nc.dram_tensor("outr", [NCH * P, KD * CW], F16, kind="ExternalOutput")

    Relu = mybir.ActivationFunctionType.Relu
    Sin = mybir.ActivationFunctionType.Sin

    with tile.TileContext(nc) as tc:
        with (
            tc.tile_pool(name="const", bufs=1) as cpool,
            tc.tile_pool(name="xin", bufs=4) as xpool,
            tc.tile_pool(name="work", bufs=2) as wpool,
            tc.tile_pool(name="stage", bufs=6) as spool,
            tc.tile_pool(name="ps1p", bufs=2, space="PSUM") as ps1pool,
            tc.tile_pool(name="psmid", bufs=2, space="PSUM") as psmpool,
            tc.tile_pool(name="psout", bufs=2, space="PSUM") as psopool,
        ):
            # ---- resident constant tiles (DMAs interleaved below) ----
            w0p_sb = cpool.tile([P, D], BF16, name="w0p_sb", tag="w0p_sb")
            w1_sb = cpool.tile([H1, H2], BF16, name="w1_sb", tag="w1_sb")
            w2g_sb = cpool.tile([H2, 81], BF16, name="w2g_sb", tag="w2g_sb")
            wfg_sb = cpool.tile([81, H2], BF16, name="wfg_sb", tag="wfg_sb")
            wd1_sb = cpool.tile([H2, H1], BF16, name="wd1_sb", tag="wd1_sb")
            wd2_sb = cpool.tile([H1, D], BF16, name="wd2_sb", tag="wd2_sb")
            bia_sb = cpool.tile([P, 21], F32, name="bia_sb", tag="bia_sb")

            # ---- x in: raw f32 chunk-major [128, 512] tiles, all on the
            # sync HWDGE ring (sync has no compute duties, so ring-full
            # stalls are harmless). Chunk 0's tiles are interleaved with
            # w0p's 16 [128,128] column blocks so L1(0) step k has both its
            # weight block and x tile as early as possible. The small
            # weights go on the scalar HWDGE ring up front: 7 triggers that
            # drain in ~2us, long before the first eviction needs the
            # scalar engine. ----
            nc.scalar.dma_start(out=bia_sb[:], in_=bia[:])
            nc.scalar.dma_start(out=w1_sb[:], in_=w1[:])
            nc.scalar.dma_start(out=w2g_sb[:], in_=w2g[:])
            nc.scalar.dma_start(out=wfg_sb[:], in_=wfg[:])
            nc.scalar.dma_start(out=wd1_sb[:], in_=wd1[:])

            nc.sync.dma_start(out=w0p_sb[:], in_=w0p[:])
            xts = {}
            for n in range(NCH):
                for q in range(2):
                    t = xpool.tile([P, 8 * CW], BF16, name=f"x_{n}_{q}", tag=f"xt{q}")
                    nc.sync.dma_start(
                        out=t[:],
                        in_=xin[n * P : (n + 1) * P, q * 8 * CW : (q + 1) * 8 * CW],
                    )
                    xts[(n, q)] = t
                if n == 1:
                    # wd2 is only needed at dec(0) (~18us): queue it on sync
                    # BEHIND chunks 0-1 so it delays late chunks, not chunk 0
                    nc.sync.dma_start(out=wd2_sb[:], in_=wd2[:])

            def x_slice(n, k):
                return xts[(n, k // 8)][:, (k % 8) * CW : (k % 8 + 1) * CW]

            state = {}

            def emit_l1(n):
                """16 accumulating f32r matmuls + relu eviction for chunk n."""
                ps1 = ps1pool.tile([P, CW], F32, name=f"ps1_{n}", tag="ps1")
                h1 = wpool.tile([P, CW], BF16, name=f"h1_{n}", tag="h1")
                state[n] = h1
                steps = []
                for k in range(KD):
                    def mm(k=k, ps1=ps1, n=n):
                        nc.tensor.matmul(
                            ps1[:],
                            w0p_sb[:, k * P : (k + 1) * P],
                            x_slice(n, k),
                            start=(k == 0),
                            stop=(k == KD - 1),
                        )
                    steps.append(mm)
                def ev(ps1=ps1, h1=h1):
                    nc.scalar.activation(h1[:], ps1[:], Relu, bias=bia_sb[:, 0:1])
                steps.append(ev)
                return steps

            def emit_mid(n):
                """Mid layers for batch chunk n (produces h4)."""
                h1 = state[n]
                steps = []

                ps2 = psmpool.tile([H2, CW], F32, name=f"ps2_{n}", tag="psm")
                h2 = wpool.tile([H2, CW], BF16, name=f"h2_{n}", tag="h2")
                steps.append(lambda: nc.tensor.matmul(ps2[:], w1_sb[:], h1[:], start=True, stop=True))
                steps.append(lambda: nc.scalar.activation(h2[:], ps2[:], Relu, bias=bia_sb[0:H2, 1:2]))

                ps3 = psmpool.tile([81, CW], F32, name=f"ps3_{n}", tag="psm")
                fst = wpool.tile([81, CW], BF16, name=f"fst_{n}", tag="fst")
                steps.append(lambda: nc.tensor.matmul(ps3[:], w2g_sb[:], h2[:], start=True, stop=True))
                steps.append(lambda: nc.scalar.activation(fst[:], ps3[:], Sin, bias=bia_sb[0:81, 2:3]))

                ps4 = psmpool.tile([H2, CW], F32, name=f"ps4_{n}", tag="psm")
                h3 = wpool.tile([H2, CW], BF16, name=f"h3_{n}", tag="h3")
                steps.append(lambda: nc.tensor.matmul(ps4[:], wfg_sb[:], fst[:], start=True, stop=True))
                steps.append(lambda: nc.scalar.activation(h3[:], ps4[:], Relu, bias=bia_sb[0:H2, 3:4]))

                ps5 = psmpool.tile([H1, CW], F32, name=f"ps5_{n}", tag="psm")
                h4 = wpool.tile([H1, CW], BF16, name=f"h4_{n}", tag="h4")
                steps.append(lambda: nc.tensor.matmul(ps5[:], wd1_sb[:], h3[:], start=True, stop=True))
                steps.append(lambda: nc.scalar.activation(h4[:], ps5[:], Relu, bias=bia_sb[0:H1, 4:5]))
                state[("h4", n)] = h4
                return steps

            def emit_dec(n):
                """Decoder head + out-DMAs for batch chunk n. Evictions
                stage 4 column blocks into one [128, 2048] f16 tile, then a
                single contiguous 512KB DMA writes the group; chunk 0's
                writes go on gpsimd (sync is still streaming x), later
                chunks on the by-then-idle sync ring."""
                h4 = state[("h4", n)]
                steps = []
                for g in range(4):
                    ost4 = spool.tile([P, 4 * CW], F16, name=f"ost_{n}_{g}", tag="ost")
                    for h in range(2):
                        # pair two column blocks in one 2-bank PSUM tile so a
                        # single 1024-wide eviction drains both
                        ps6 = psopool.tile([P, 2 * CW], F32, name=f"ps6_{n}_{g}_{h}", tag="pso")
                        for j in range(2):
                            mg = 4 * g + 2 * h + j
                            def mm6(ps6=ps6, j=j, mg=mg, h4=h4):
                                nc.tensor.matmul(
                                    ps6[:, j * CW : (j + 1) * CW],
                                    wd2_sb[:, mg * P : (mg + 1) * P],
                                    h4[:],
                                    start=True,
                                    stop=True,
                                )
                            steps.append(mm6)
                        dst = ost4[:, 2 * h * CW : 2 * (h + 1) * CW]
                        use_dve = (g + h + n) % 2 == 0
                        if dec2_bias_zero:
                            if use_dve:
                                def ev6(dst=dst, ps6=ps6):
                                    nc.vector.tensor_copy(dst, ps6[:])
                            else:
                                def ev6(dst=dst, ps6=ps6):
                                    nc.scalar.copy(dst, ps6[:])
                            steps.append(ev6)
                        else:
                            mg0 = 4 * g + 2 * h
                            if use_dve:
                                def ev6(dst=dst, ps6=ps6, mg0=mg0):
                                    for j in range(2):
                                        nc.vector.tensor_scalar_add(
                                            dst[:, j * CW : (j + 1) * CW],
                                            ps6[:, j * CW : (j + 1) * CW],
                                            bia_sb[:, 5 + mg0 + j : 6 + mg0 + j],
                                        )
                            else:
                                def ev6(dst=dst, ps6=ps6, mg0=mg0):
                                    for j in range(2):
                                        nc.scalar.add(
                                            dst[:, j * CW : (j + 1) * CW],
                                            ps6[:, j * CW : (j + 1) * CW],
                                            bia_sb[:, 5 + mg0 + j : 6 + mg0 + j],
                                        )
                            steps.append(ev6)
                    def dma6(ost4=ost4, g=g, n=n):
                        if n == NCH - 1:
                            # final chunk: split each write across both rings
                            # so the tail drains two-wide
                            for h2_, eng in ((0, nc.sync), (1, nc.gpsimd)):
                                eng.dma_start(
                                    out=outr[
                                        n * P : (n + 1) * P,
                                        (2 * g + h2_) * 2 * CW : (2 * g + h2_ + 1) * 2 * CW,
                                    ],
                                    in_=ost4[:, h2_ * 2 * CW : (h2_ + 1) * 2 * CW],
                                )
                        else:
                            eng = nc.gpsimd if n <= 1 else nc.sync
                            eng.dma_start(
                                out=outr[n * P : (n + 1) * P, g * 4 * CW : (g + 1) * 4 * CW],
                                in_=ost4[:],
                            )
                    steps.append(dma6)
                return steps

            def ratio_merge(*streams):
                streams = [s for s in streams if s]
                out = []
                idx = [0] * len(streams)
                total = sum(len(s) for s in streams)
                for _ in range(total):
                    best, bestv = None, None
                    for si, s in enumerate(streams):
                        if idx[si] >= len(s):
                            continue
                        v = idx[si] / len(s)
                        if bestv is None or v < bestv:
                            best, bestv = si, v
                    out.append(streams[best][idx[best]])
                    idx[best] += 1
                return out

            # preload the Sin ACT table set while the Scalar engine is idle
            dsin = cpool.tile([1, 8], F32, name="dsin", tag="dsin")
            nc.vector.memset(dsin[:], 0.0)
            nc.scalar.activation(dsin[:, 4:8], dsin[:, 0:4], Sin)

            # PE warm-up: short narrow dummy matmuls warm the HAM clock gate
            # while w0p + the first x piece stream in, without delaying L1(0)
            # by more than ~2us (128-wide MMs are ~160ns even cold).
            wtile = cpool.tile([P, CW], BF16, name="wtile", tag="wtile")
            nc.vector.memset(wtile[:], 0.0)
            if warmup:
                wps = psopool.tile([P, 128], F32, name="wps", tag="pso")
                for i in range(warmup):
                    nc.tensor.matmul(
                        wps[:], wtile[:, 0:128], wtile[:, 0:128],
                        start=(i == 0), stop=(i == warmup - 1),
                    )

            # software pipeline: L1(n) interleaved with mid(n-1) followed by
            # dec(n-1) (one-chunk latency so writes start early); small dummy
            # bursts in n=1..2 bridge the x-starvation gaps and keep HAM warm
            for n in range(NCH + 1):
                if n in (1, 2) and warmup2:
                    wpsn = psopool.tile([P, CW], F32, name=f"wps{n}", tag="pso")
                    for i in range(warmup2):
                        nc.tensor.matmul(
                            wpsn[:], wtile[:, 0:128], wtile[:], start=(i == 0),
                            stop=(i == warmup2 - 1),
                        )
                a = emit_l1(n) if n < NCH else []
                if 1 <= n <= NCH:
                    mid_steps = emit_mid(n - 1)
                    dec_steps = emit_dec(n - 1)
                    if n == NCH - 1:
                        # hold back the tail of dec(NCH-2): in the final
                        # iteration L1 is empty, so these independent MMs
                        # cover mid(NCH-1)'s ACT round-trip stalls
                        held = dec_steps[-10:]
                        dec_steps = dec_steps[:-10]
                    if n == NCH:
                        b = ratio_merge(mid_steps, held) + dec_steps
                    else:
                        b = mid_steps + dec_steps
                else:
                    b = []
                for step in ratio_merge(a, b):
                    step()

    nc.compile()
    _PROGRAM_CACHE[key] = nc
    return nc


# =====================================================================
# Host wrapper: shard, run, gather
# =====================================================================


def make_in_maps(
    x, enc_w0, enc_b0, enc_w1, enc_b1, enc_w2, enc_b2, qw,
    dec_w0, dec_b0, dec_w1, dec_b1, dec_w2, dec_b2,
):
    f32 = np.float32
    # circuit collapsed to the Fourier feature basis, folded into dec_w0
    w2g_h, biasg, wfg_h = _fourier_mid(qw, enc_w2, enc_b2, dec_w0)

    # enc_w0 repacked so SBUF col-block k holds rows k*128..(k+1)*128
    w0p = (
        np.asarray(enc_w0, f32).reshape(KD, P, H1).transpose(1, 0, 2).reshape(P, D)
    )
    w0p = np.ascontiguousarray(w0p)

    bia = np.zeros((P, 21), dtype=f32)
    bia[:, 0] = enc_b0
    bia[:H2, 1] = enc_b1
    bia[:81, 2] = biasg  # Fourier stack phases (incl. enc_b2 contribution)
    bia[:H2, 3] = dec_b0
    bia[:H1, 4] = dec_b1
    bia[:, 5 : 5 + KD] = np.asarray(dec_b2, f32).reshape(KD, P).T

    bf16 = ml_dtypes.bfloat16
    common = {
        "w0p": w0p.astype(bf16),
        "w1": np.ascontiguousarray(np.asarray(enc_w1, f32)).astype(bf16),
        "w2g": np.ascontiguousarray(w2g_h).astype(bf16),
        "wfg": np.ascontiguousarray(wfg_h).astype(bf16),
        "wd1": np.ascontiguousarray(np.asarray(dec_w1, f32)).astype(bf16),
        "wd2": np.ascontiguousarray(np.asarray(dec_w2, f32)).astype(bf16),
        "bia": bia,
    }

    # repack x per core: xin[n*128+p, k*512+c] = x[c0 + n*512 + c, k*128+p]
    # cast to bf16 on host (same rounding the device DMA cast would do)
    xr = np.ascontiguousarray(
        np.asarray(x, f32)
        .reshape(NCORES, NCH, CW, KD, P)
        .transpose(0, 1, 4, 3, 2)
    ).astype(bf16).reshape(NCORES, NCH * P, KD * CW)
    in_maps = []
    for c in range(NCORES):
        m = dict(common)
        m["xin"] = xr[c]
        in_maps.append(m)
    return in_maps


def gather_output(results):
    # outr[n*128+p, k*512+c] = out[c0 + n*512 + c, k*128 + p]
    arr = np.stack([results[c]["outr"] for c in range(NCORES)])
    arr = arr.reshape(NCORES, NCH, P, KD, CW).transpose(0, 1, 4, 3, 2)
    return np.ascontiguousarray(arr).reshape(B, D).astype(np.float32)


def kernel(**inputs):
    nc = _build_program(
        dec2_bias_zero=not np.any(np.asarray(inputs["dec_b2"], np.float32))
    )
    in_maps = make_in_maps(**inputs)
    res = run_bass_kernel_spmd(nc, in_maps, core_ids=list(range(NCORES)))
    return gather_output(res.results)


if __name__ == "__main__":
    rng = np.random.default_rng(0)
    demo = {
        "x": rng.normal(size=(B, D)).astype(np.float32),
        "enc_w0": rng.normal(size=(D, H1)).astype(np.float32) * 0.02,
        "enc_b0": np.zeros(H1, np.float32),
        "enc_w1": rng.normal(size=(H1, H2)).astype(np.float32) * 0.02,
        "enc_b1": np.zeros(H2, np.float32),
        "enc_w2": rng.normal(size=(H2, L)).astype(np.float32) * 0.02,
        "enc_b2": np.zeros(L, np.float32),
        "qw": rng.normal(size=(NL, NQ, 3)).astype(np.float32),
        "dec_w0": rng.normal(size=(NQ, H2)).astype(np.float32) * 0.02,
        "dec_b0": np.zeros(H2, np.float32),
        "dec_w1": rng.normal(size=(H2, H1)).astype(np.float32) * 0.02,
        "dec_w2": rng.normal(size=(H1, D)).astype(np.float32) * 0.02,
        "dec_b1": np.zeros(H1, np.float32),
        "dec_b2": np.zeros(D, np.float32),
    }
    out = kernel(**demo)
    print("kernel ran, out shape:", out.shape, "finite:", np.isfinite(out).all())
